# revision 15
# baseline (speedup 1.0000x reference)
"""BotGCN single-chip Trainium2 kernel (8 NeuronCores, SPMD + collectives).

Strategy (graph/data parallel, per sharding hint):
  - Nodes sharded 6250/core, padded to 6272 = 49 tiles of 128. A per-core
    node permutation balances incoming-edge counts across the 49 dst blocks.
  - Host preprocessing (indices only): permute/pad nodes, partition edges by
    (dst core, dst block, src half), compute degrees, pack gather indices
    (int16, wrapped, -1 padded) + per-group exact counts + dst-local one-hot
    keys.
  - Device per core: fused input projections (P->I, no DRAM bounce) ->
    layer transform X@W (feature-major lhsT), row scale by dinv -> AllGather
    of Y = dinv*(XW) (bf16) -> aggregation: per (dst block, src half)
    dma_gather with PREPARE_ONLY desc-gen pipelined against trigger+DMA
    drain (desc count = per-core exact edge count via num_idxs_reg), one-hot
    (iota==key) matmuls accumulate messages in PSUM (+ self-loop + bias) ->
    dinv scale -> PE transpose to feature-major SBUF tiles -> next layer's
    transform (or the output head) inlined per block under the gather shadow.
"""

import os
import sys

if "/opt/trn_rl_repo" not in sys.path:
    sys.path.insert(0, "/opt/trn_rl_repo")

import numpy as np

import concourse.bacc as bacc
import concourse.bass as bass
import concourse.mybir as mybir
import concourse.tile as tile
from concourse import library_config
from concourse.bass_utils import run_bass_kernel_spmd

# ---------------- problem constants ----------------
N = 50000
E = 800000
NCORE = 8
NPC = N // NCORE            # 6250 nodes per core
BLK = 49                    # dst blocks per core
NPAD = BLK * 128            # 6272 padded nodes per core
GPAD = NCORE * NPAD         # 50176 padded global nodes
HALF = GPAD // 2            # 25088 (int16-addressable gather halves)
DES = 768
F = 384                     # embedding dim
NT_W = 448                  # wide node tile (14 per core)
NW = NPAD // NT_W           # 14
CAPT = 9                    # gather tiles per (block, half-chunk)
CAP = CAPT * 128            # 1152 edge slots per (block, chunk)
IDXC = CAP // 16            # 72 idx columns per (block, chunk)
NGRP = BLK * 2              # 98 gather groups per core per layer
AHEAD = 4                   # gather desc-gen prep lookahead

FP32 = mybir.dt.float32

# matmul storage mode: "bf16" (default) or "f32r"
MM_MODE = os.environ.get("MM_MODE", "bf16")
# gather mode: "direct_reg" (two plain calls, exact per-core counts via
# register), "direct" (two plain calls, static full counts, 0-padded idx),
# "prep_reg" (prepare_only + trigger pipeline, single 1152 call)
GMODE = os.environ.get("GMODE", "direct_reg")

_CACHED = {}

if MM_MODE == "bf16":
    MDT = mybir.dt.bfloat16
elif MM_MODE == "f32r":
    MDT = mybir.dt.float32r
else:
    MDT = FP32


# ---------------- host preprocessing ----------------

def _balance_blocks(deg_lo, deg_hi):
    """Assign NPC nodes to BLK bins (cap 128 nodes, cap CAP per chunk).
    Returns pos[node] in [0, NPAD). Greedy: heaviest nodes first."""
    order = np.argsort(-(deg_lo + deg_hi), kind="stable")
    cnt = np.zeros(BLK, np.int64)
    lo = np.zeros(BLK, np.int64)
    hi = np.zeros(BLK, np.int64)
    pos = np.empty(NPC, np.int64)
    for n in order:
        dlo, dhi = deg_lo[n], deg_hi[n]
        feas = (cnt < 128) & (lo + dlo <= CAP) & (hi + dhi <= CAP)
        assert feas.any(), "block balancing failed; raise CAPT"
        score = np.where(feas, np.maximum(lo + dlo, hi + dhi) * 1000 + cnt,
                         1 << 60)
        b = int(np.argmin(score))
        pos[n] = b * 128 + cnt[b]
        cnt[b] += 1
        lo[b] += dlo
        hi[b] += dhi
    return pos


def _preprocess(edge_index):
    """All index-space preprocessing. Returns per-core packed index arrays,
    exact per-group counts, degree tensors, and the node permutation."""
    src = np.asarray(edge_index[0], np.int64)
    dst = np.asarray(edge_index[1], np.int64)
    deg = np.bincount(dst, minlength=N).astype(np.int64)

    src_core = src // NPC
    # chunk of an edge = which gather half its (padded) src lands in.
    e_chunk = (src_core >= NCORE // 2).astype(np.int64)

    deg_lo_all = np.bincount(dst[e_chunk == 0], minlength=N)
    deg_hi_all = np.bincount(dst[e_chunk == 1], minlength=N)

    pad_pos = np.empty(N, np.int64)  # node -> padded global position
    for c in range(NCORE):
        sl = slice(c * NPC, (c + 1) * NPC)
        pos = _balance_blocks(deg_lo_all[sl], deg_hi_all[sl])
        pad_pos[sl] = c * NPAD + pos

    sp = pad_pos[src]
    dp = pad_pos[dst]
    e_core = dp // NPAD
    e_block = (dp % NPAD) // 128
    e_dl = dp % 128
    e_idx16 = sp - e_chunk * HALF
    assert e_idx16.max() < HALF and e_idx16.min() >= 0

    # group edges by (core, block, chunk), sort by src for DMA locality
    order = np.lexsort((e_idx16, e_chunk, e_block, e_core))
    g_core = e_core[order]
    g_block = e_block[order]
    g_chunk = e_chunk[order]
    g_idx = e_idx16[order]
    g_dl = e_dl[order]

    gid = (g_core * BLK + g_block) * 2 + g_chunk
    ngroups = NCORE * NGRP
    counts = np.bincount(gid, minlength=ngroups)
    assert counts.max() <= CAP, f"group overflow {counts.max()} > {CAP}"
    starts = np.zeros(ngroups, np.int64)
    np.cumsum(counts[:-1], out=starts[1:])
    slot_in_g = np.arange(len(gid)) - starts[gid]

    pad_idx = 0 if GMODE == "direct" else -1
    idx_slots = np.full((ngroups, CAP), pad_idx, np.int16)
    dl_slots = np.full((ngroups, CAP), 999.0, np.float32)  # pad -> no match
    idx_slots[gid, slot_in_g] = g_idx.astype(np.int16)
    dl_slots[gid, slot_in_g] = g_dl.astype(np.float32)

    cnt = counts.reshape(NCORE, NGRP).astype(np.int64)
    if GMODE == "prep_reg":
        # single 1152-idx call: count >= 1 (force one valid slot if empty)
        for g in np.nonzero(counts == 0)[0]:
            idx_slots[g, 0] = 0
        gcnt1 = np.maximum(cnt, 1).astype(np.uint32)
        gcnt2 = np.zeros_like(gcnt1)
    else:
        # two calls: window1 = slots [0,1024), window2 = [1024,1152).
        # window2 needs >= 1 valid idx so its reg count is never 0.
        for g in np.nonzero(counts <= 1024)[0]:
            idx_slots[g, 1024] = 0
        for g in np.nonzero(counts == 0)[0]:
            idx_slots[g, 0] = 0
        gcnt1 = np.maximum(np.minimum(cnt, 1024), 1).astype(np.uint32)
        gcnt2 = np.maximum(cnt - 1024, 1).astype(np.uint32)

    per_core = []
    for c in range(NCORE):
        gs = idx_slots[c * NGRP:(c + 1) * NGRP]           # [98, CAP]
        ds = dl_slots[c * NGRP:(c + 1) * NGRP]            # [98, CAP]
        # idx16 wrapped: slot j at [j%16, j//16], tiled x8 on partitions
        w = gs.reshape(NGRP, IDXC, 16).transpose(2, 0, 1).reshape(
            16, NGRP * IDXC)
        idx16 = np.tile(w, (8, 1)).copy()                 # [128, 98*72]
        # dst-local wrapped per tile: slot j at [j%128, j//128]
        dstl = ds.reshape(NGRP, CAPT, 128).transpose(2, 0, 1).reshape(
            128, NGRP * CAPT).copy()                      # [128, 98*9]
        per_core.append((idx16, dstl,
                         np.stack([gcnt1[c], gcnt2[c]]).copy()))

    # per-core degree tensors in padded-position order
    deg1_col = np.ones((NCORE, 128, BLK), np.float32)
    for c in range(NCORE):
        p = pad_pos[c * NPC:(c + 1) * NPC] - c * NPAD
        d1 = (deg[c * NPC:(c + 1) * NPC] + 1).astype(np.float32)
        deg1_col[c, p % 128, p // 128] = d1

    return pad_pos, per_core, deg1_col


# ---------------- device program ----------------

def _build():
    nc = bacc.Bacc("TRN2", target_bir_lowering=False, num_devices=NCORE)
    dt_in = MDT

    def ein(name, shape, dt=dt_in):
        return nc.dram_tensor(name, shape, dt, kind="ExternalInput")

    desT = ein("desT", [DES, NPAD])
    numT = ein("numT", [4, NPAD])
    catT = ein("catT", [3, NPAD])
    w_des = ein("w_des", [128, 6, 128])
    w_num = ein("w_num", [4, 128])
    w_cat = ein("w_cat", [3, 128])
    w_in = ein("w_in", [128, 3, F])
    w_g1 = ein("w_g1", [128, 3, F])
    w_g2 = ein("w_g2", [128, 3, F])
    w_o1 = ein("w_o1", [128, 3, F])
    w_o2 = ein("w_o2", [128, 3, 2])
    b_des = ein("b_des", [128, 1], FP32)
    b_num = ein("b_num", [128, 1], FP32)
    b_cat = ein("b_cat", [128, 1], FP32)
    b_in = ein("b_in", [128, 3], FP32)
    b_g1 = ein("b_g1", [1, F], FP32)
    b_g2 = ein("b_g2", [1, F], FP32)
    b_o1 = ein("b_o1", [128, 3], FP32)
    b_o2 = ein("b_o2", [2, 1], FP32)
    deg1c = ein("deg1c", [128, BLK], FP32)
    idx16 = ein("idx16", [128, NGRP * IDXC], mybir.dt.int16)
    dstl = ein("dstl", [128, NGRP * CAPT], FP32)
    gcnt = ein("gcnt", [2, NGRP], mybir.dt.uint32)

    out2 = nc.dram_tensor("out2", [2, NPAD], FP32, kind="ExternalOutput")

    xin = nc.dram_tensor("xin", [128, 3, NPAD], dt_in)
    yown = [nc.dram_tensor(f"y{l}own", [NPAD, F], dt_in) for l in (1, 2)]
    yall = [nc.dram_tensor(f"yall{l}", [GPAD, F], dt_in, addr_space="Shared")
            for l in (1, 2)]

    LR = mybir.ActivationFunctionType.Lrelu
    CP = mybir.ActivationFunctionType.Copy
    SQ = mybir.ActivationFunctionType.Sqrt
    EQ = mybir.AluOpType.is_equal

    with tile.TileContext(nc) as tc:
        with (
            tc.tile_pool(name="cst", bufs=1) as cst,
            tc.tile_pool(name="hfp", bufs=1) as hfp,
            tc.tile_pool(name="wide", bufs=8) as wide,
            tc.tile_pool(name="nar", bufs=6) as nar,
            tc.tile_pool(name="gp", bufs=AHEAD + 1) as gp,
            tc.tile_pool(name="oh", bufs=3) as ohp,
            tc.tile_pool(name="pw", bufs=2, space="PSUM") as pw,
            tc.tile_pool(name="pa", bufs=2, space="PSUM") as pa,
            tc.tile_pool(name="pt", bufs=2, space="PSUM") as pt,
        ):
            nc.gpsimd.load_library(library_config.mlp)

            # ---- constants in SBUF
            iotab = cst.tile([128, CAPT, 128], FP32)
            nc.gpsimd.iota(iotab[:], pattern=[[0, CAPT], [1, 128]], base=0,
                           channel_multiplier=0,
                           allow_small_or_imprecise_dtypes=True)
            pcol = cst.tile([128, 1], FP32)
            nc.gpsimd.iota(pcol[:], pattern=[[0, 1]], base=0,
                           channel_multiplier=1,
                           allow_small_or_imprecise_dtypes=True)
            iden = cst.tile([128, 128], dt_in)
            nc.vector.tensor_scalar(out=iden[:], in0=iotab[:, 0, :],
                                    scalar1=pcol[:, 0:1], scalar2=None,
                                    op0=mybir.AluOpType.is_equal)
            idx_sb = cst.tile([128, NGRP * IDXC], mybir.dt.int16)
            nc.sync.dma_start(idx_sb[:], idx16.ap())
            dstl_sb = cst.tile([128, NGRP * CAPT], FP32)
            nc.sync.dma_start(dstl_sb[:], dstl.ap())
            gcnt_sb = cst.tile([2, NGRP], mybir.dt.uint32)
            nc.sync.dma_start(gcnt_sb[:], gcnt.ap())

            wdes_sb = cst.tile([128, 6, 128], dt_in)
            nc.sync.dma_start(wdes_sb[:], w_des.ap())
            wnum_sb = cst.tile([4, 128], dt_in)
            nc.sync.dma_start(wnum_sb[:], w_num.ap())
            wcat_sb = cst.tile([3, 128], dt_in)
            nc.sync.dma_start(wcat_sb[:], w_cat.ap())
            win_sb = cst.tile([128, 3, F], dt_in)
            nc.sync.dma_start(win_sb[:], w_in.ap())
            wg1_sb = cst.tile([128, 3, F], dt_in)
            nc.sync.dma_start(wg1_sb[:], w_g1.ap())
            wg2_sb = cst.tile([128, 3, F], dt_in)
            nc.sync.dma_start(wg2_sb[:], w_g2.ap())
            wo1_sb = cst.tile([128, 3, F], dt_in)
            nc.sync.dma_start(wo1_sb[:], w_o1.ap())
            wo2_sb = cst.tile([128, 3, 2], dt_in)
            nc.sync.dma_start(wo2_sb[:], w_o2.ap())

            bdes_sb = cst.tile([128, 1], FP32)
            nc.sync.dma_start(bdes_sb[:], b_des.ap())
            bnum_sb = cst.tile([128, 1], FP32)
            nc.sync.dma_start(bnum_sb[:], b_num.ap())
            bcat_sb = cst.tile([128, 1], FP32)
            nc.sync.dma_start(bcat_sb[:], b_cat.ap())
            bin_sb = cst.tile([128, 3], FP32)
            nc.sync.dma_start(bin_sb[:], b_in.ap())
            bg_sb = [cst.tile([1, F], FP32, tag=f"bg{l}", name=f"bg{l}")
                     for l in (0, 1)]
            nc.sync.dma_start(bg_sb[0][:], b_g1.ap())
            nc.sync.dma_start(bg_sb[1][:], b_g2.ap())
            bo1_sb = cst.tile([128, 3], FP32)
            nc.sync.dma_start(bo1_sb[:], b_o1.ap())
            bo2_sb = cst.tile([2, 1], FP32)
            nc.sync.dma_start(bo2_sb[:], b_o2.ap())

            d1c_sb = cst.tile([128, BLK], FP32)
            nc.sync.dma_start(d1c_sb[:], deg1c.ap())
            tmp_c = cst.tile([128, BLK], FP32)
            nc.vector.reciprocal(tmp_c[:], d1c_sb[:])
            dinv_c = cst.tile([128, BLK], FP32)
            nc.scalar.activation(dinv_c[:], tmp_c[:], SQ)
            # replicate gcn biases across partitions: brep[l] = ones x b_g
            ones_r = cst.tile([1, 128], FP32)
            nc.vector.memset(ones_r[:], 1.0)
            brep = []
            for l in (0, 1):
                psb = pt.tile([128, F], FP32, space="PSUM", tag="pbr",
                              name=f"psb{l}", bufs=1)
                nc.tensor.matmul(psb[:], lhsT=ones_r[:], rhs=bg_sb[l][:],
                                 start=True, stop=True)
                br = cst.tile([128, F], FP32, name=f"brep{l}")
                nc.vector.tensor_copy(br[:], psb[:])
                brep.append(br)

            # zero-fill the gather ring buffers once (stale tails feed
            # one-hot zeros; must be finite)
            for _ in range(AHEAD + 1):
                gz = gp.tile([128, CAPT, F], dt_in, tag="gath")
                nc.vector.memset(gz[:], 0.0)

            dma_sem = nc.alloc_semaphore("gdma")
            cnt_reg = nc.gpsimd.alloc_register("gcnt_reg")
            cnt_reg2 = nc.gpsimd.alloc_register("gcnt_reg2")

            # ---- fused phase P+I: input projections -> xin (feature-major)
            for t in range(NW):
                ns = bass.ts(t, NT_W)
                ps_d = pw.tile([128, NT_W], FP32, space="PSUM", tag="pwide")
                for k in range(6):
                    r = wide.tile([128, NT_W], dt_in, tag="wrhs")
                    nc.sync.dma_start(r[:], desT.ap()[bass.ts(k, 128), ns])
                    nc.tensor.matmul(ps_d[:], lhsT=wdes_sb[:, k, :],
                                     rhs=r[:], start=(k == 0), stop=(k == 5))
                o_d = nar.tile([128, NT_W], dt_in, tag="mid")
                nc.scalar.activation(o_d[:], ps_d[:], LR, bias=bdes_sb[:, 0:1],
                                     alpha=0.01)

                r_n = wide.tile([4, NT_W], dt_in, tag="wrhs")
                nc.sync.dma_start(r_n[:], numT.ap()[:, ns])
                ps_n = pw.tile([128, NT_W], FP32, space="PSUM", tag="pwide")
                nc.tensor.matmul(ps_n[:], lhsT=wnum_sb[:], rhs=r_n[:],
                                 start=True, stop=True)
                o_n = nar.tile([128, NT_W], dt_in, tag="mid")
                nc.scalar.activation(o_n[:], ps_n[:], LR, bias=bnum_sb[:, 0:1],
                                     alpha=0.01)

                r_c = wide.tile([3, NT_W], dt_in, tag="wrhs")
                nc.sync.dma_start(r_c[:], catT.ap()[:, ns])
                ps_c = pw.tile([128, NT_W], FP32, space="PSUM", tag="pwide")
                nc.tensor.matmul(ps_c[:], lhsT=wcat_sb[:], rhs=r_c[:],
                                 start=True, stop=True)
                o_c = nar.tile([128, NT_W], dt_in, tag="mid")
                nc.scalar.activation(o_c[:], ps_c[:], LR, bias=bcat_sb[:, 0:1],
                                     alpha=0.01)

                rs = [o_d, o_n, o_c]
                for m in range(3):
                    ps = pw.tile([128, NT_W], FP32, space="PSUM", tag="pwide")
                    for k in range(3):
                        nc.tensor.matmul(
                            ps[:], lhsT=win_sb[:, k, bass.ts(m, 128)],
                            rhs=rs[k][:], start=(k == 0), stop=(k == 2))
                    o = nar.tile([128, NT_W], dt_in, tag="mid")
                    nc.scalar.activation(o[:], ps[:], LR, bias=bin_sb[:, m:m + 1],
                                         alpha=0.01)
                    nc.sync.dma_start(xin.ap()[:, m, ns], o[:])

            # ---- layer-1 transform: y1 = dinv * (x @ Wg1)
            for t in range(BLK):
                ns = bass.ts(t, 128)
                ps = pa.tile([128, F], FP32, space="PSUM", tag="pagg")
                for k in range(3):
                    lx = nar.tile([128, 128], dt_in, tag="lx")
                    nc.sync.dma_start(lx[:], xin.ap()[:, k, ns])
                    nc.tensor.matmul(ps[:], lhsT=lx[:], rhs=wg1_sb[:, k, :],
                                     start=(k == 0), stop=(k == 2))
                y_t = nar.tile([128, F], dt_in, tag="mid")
                nc.scalar.activation(y_t[:], ps[:], CP,
                                     scale=dinv_c[:, t:t + 1])
                nc.sync.dma_start(yown[0].ap()[ns, :], y_t[:])

            # ---- gather emit helper (mode-dependent)
            def emit_gather(li, i):
                ya = yall[li]
                ch = i % 2
                g = gp.tile([128, CAPT, F], dt_in, tag="gath")
                src = ya.ap()[ch * HALF:(ch + 1) * HALF, :]
                c0 = i * IDXC
                if GMODE == "prep_reg":
                    nc.gpsimd.reg_load(cnt_reg, gcnt_sb[0:1, i:i + 1])
                    nc.gpsimd.dma_gather(
                        g[:, 0:CAPT, :], src, idx_sb[:, c0:c0 + IDXC],
                        CAP, cnt_reg, F, prepare_only=True, sem=dma_sem)
                elif GMODE == "direct_reg":
                    nc.gpsimd.reg_load(cnt_reg, gcnt_sb[0:1, i:i + 1])
                    nc.gpsimd.dma_gather(
                        g[:, 0:8, :], src, idx_sb[:, c0:c0 + 64],
                        1024, cnt_reg, F)
                    nc.gpsimd.reg_load(cnt_reg2, gcnt_sb[1:2, i:i + 1])
                    nc.gpsimd.dma_gather(
                        g[:, 8:CAPT, :], src, idx_sb[:, c0 + 64:c0 + IDXC],
                        CAP - 1024, cnt_reg2, F)
                else:  # "direct"
                    nc.gpsimd.dma_gather(
                        g[:, 0:8, :], src, idx_sb[:, c0:c0 + 64],
                        1024, 1024, F)
                    nc.gpsimd.dma_gather(
                        g[:, 8:CAPT, :], src, idx_sb[:, c0 + 64:c0 + IDXC],
                        CAP - 1024, CAP - 1024, F)
                return g

            # ---- two GCN layers: AllGather + aggregation (+T2/head inline)
            for li in range(2):
                yo = yown[li]
                ya = yall[li]
                bg = brep[li]

                gtiles = [None] * NGRP
                npend = 0
                if GMODE == "prep_reg":
                    for j in range(AHEAD):
                        gtiles[j] = emit_gather(li, j)
                        npend += 1
                nc.gpsimd.collective_compute(
                    "AllGather", mybir.AluOpType.bypass,
                    replica_groups=[list(range(NCORE))],
                    ins=[yo.ap()], outs=[ya.ap()])

                ps = None
                for i in range(NGRP):
                    b, ch = divmod(i, 2)
                    if GMODE == "prep_reg":
                        if npend:
                            nc.gpsimd.trigger_dma(count=None)
                            npend = 0
                        g = gtiles[i]
                    else:
                        g = emit_gather(li, i)
                    if ch == 0:
                        ps = pa.tile([128, F], FP32, space="PSUM", tag="pagg")
                    oh = ohp.tile([128, CAPT, 128], dt_in, tag="onehot")
                    dsl = dstl_sb[:, i * CAPT:(i + 1) * CAPT]
                    dsl_b = bass.AP(dsl.tensor, dsl.offset,
                                    list(dsl.ap) + [[0, 128]])
                    nc.vector.tensor_tensor(
                        out=oh[:], in0=iotab[:], in1=dsl_b, op=EQ)
                    for t in range(CAPT):
                        nc.tensor.matmul(ps[:], lhsT=oh[:, t, :],
                                         rhs=g[:, t, :],
                                         start=(ch == 0 and t == 0),
                                         stop=(ch == 1 and t == CAPT - 1))
                    if GMODE == "prep_reg" and i + AHEAD < NGRP:
                        gtiles[i + AHEAD] = emit_gather(li, i + AHEAD)
                        npend += 1
                    if ch != 1:
                        continue
                    # ---- post-process block b
                    yo_t = nar.tile([128, F], dt_in, tag="mid")
                    nc.sync.dma_start(yo_t[:], yo.ap()[bass.ts(b, 128), :])
                    s1 = nar.tile([128, F], FP32, tag="mid")
                    nc.vector.tensor_tensor(out=s1[:], in0=ps[:], in1=yo_t[:],
                                            op=mybir.AluOpType.add)
                    s2 = nar.tile([128, F], FP32, tag="mid")
                    nc.scalar.activation(s2[:], s1[:], CP,
                                         scale=dinv_c[:, b:b + 1])
                    h_t = nar.tile([128, F], dt_in, tag="mid")
                    nc.vector.tensor_tensor(out=h_t[:], in0=s2[:], in1=bg[:],
                                            op=mybir.AluOpType.add)
                    # transpose to feature-major (SBUF-resident)
                    hf = hfp.tile([128, 3, 128], dt_in, tag=f"hf{li}_{b}",
                                  name=f"hf{li}_{b}")
                    for k in range(3):
                        pst = pt.tile([128, 128], dt_in, space="PSUM",
                                      tag="ptr")
                        nc.tensor.transpose(pst[:], h_t[:, bass.ts(k, 128)],
                                            iden[:])
                        nc.vector.tensor_copy(hf[:, k, :], pst[:])

                    if li == 0:
                        # layer-2 transform for block b, inlined
                        ps2 = pw.tile([128, F], FP32, space="PSUM",
                                      tag="pwide")
                        for k in range(3):
                            nc.tensor.matmul(ps2[:], lhsT=hf[:, k, :],
                                             rhs=wg2_sb[:, k, :],
                                             start=(k == 0), stop=(k == 2))
                        y2_t = nar.tile([128, F], dt_in, tag="mid")
                        nc.scalar.activation(y2_t[:], ps2[:], CP,
                                             scale=dinv_c[:, b:b + 1])
                        nc.sync.dma_start(yown[1].ap()[bass.ts(b, 128), :],
                                          y2_t[:])
                    else:
                        # output head for block b, inlined
                        o1s = []
                        for m in range(3):
                            psh = pw.tile([128, 128], FP32, space="PSUM",
                                          tag="pwide")
                            for k in range(3):
                                nc.tensor.matmul(
                                    psh[:],
                                    lhsT=wo1_sb[:, k, bass.ts(m, 128)],
                                    rhs=hf[:, k, :],
                                    start=(k == 0), stop=(k == 2))
                            o = nar.tile([128, 128], dt_in, tag="mid")
                            nc.scalar.activation(o[:], psh[:], LR,
                                                 bias=bo1_sb[:, m:m + 1],
                                                 alpha=0.01)
                            o1s.append(o)
                        psf = pt.tile([2, 128], FP32, space="PSUM",
                                      tag="pfin", bufs=1)
                        for k in range(3):
                            nc.tensor.matmul(psf[:], lhsT=wo2_sb[:, k, :],
                                             rhs=o1s[k][:],
                                             start=(k == 0), stop=(k == 2))
                        of = nar.tile([2, 128], FP32, tag="mid")
                        nc.scalar.activation(
                            of[:], psf[:],
                            mybir.ActivationFunctionType.Identity,
                            bias=bo2_sb[:, 0:1])
                        nc.sync.dma_start(out2.ap()[:, bass.ts(b, 128)],
                                          of[:])

    nc.compile()
    return nc


# ---------------- top level ----------------

def _np(x, dt=np.float32):
    return np.ascontiguousarray(np.asarray(x), dtype=dt)


def prepare(des, tweet, num_prop, cat_prop, edge_index,
            W_des, b_des, W_num, b_num, W_cat, b_cat, W_in, b_in,
            W_g1, b_g1, W_g2, b_g2, W_o1, b_o1, W_o2, b_o2):
    """Build (or fetch cached) device program + per-core input maps."""
    try:
        import ml_dtypes
        bf16 = ml_dtypes.bfloat16
    except ImportError:
        bf16 = np.float32
    mdt = bf16 if MM_MODE == "bf16" else np.float32

    ek = tuple(np.asarray(edge_index).reshape(-1)[:16].tolist())
    if "prep" not in _CACHED or _CACHED.get("ekey") != ek:
        _CACHED["prep"] = _preprocess(edge_index)
        _CACHED["ekey"] = ek
    pad_pos, per_core, deg1_col = _CACHED["prep"]

    if "nc" not in _CACHED:
        _CACHED["nc"] = _build()
    nc = _CACHED["nc"]

    des = _np(des)
    num_prop = _np(num_prop)
    cat_prop = _np(cat_prop)

    shared = dict(
        w_des=_np(W_des, mdt).reshape(6, 128, 128).transpose(1, 0, 2).copy(),
        w_num=_np(W_num, mdt), w_cat=_np(W_cat, mdt),
        w_in=_np(W_in, mdt).reshape(3, 128, F).transpose(1, 0, 2).copy(),
        w_g1=_np(W_g1, mdt).reshape(3, 128, F).transpose(1, 0, 2).copy(),
        w_g2=_np(W_g2, mdt).reshape(3, 128, F).transpose(1, 0, 2).copy(),
        w_o1=_np(W_o1, mdt).reshape(3, 128, F).transpose(1, 0, 2).copy(),
        w_o2=_np(W_o2, mdt).reshape(3, 128, 2).transpose(1, 0, 2).copy(),
        b_des=_np(b_des).reshape(128, 1), b_num=_np(b_num).reshape(128, 1),
        b_cat=_np(b_cat).reshape(128, 1),
        b_in=_np(b_in).reshape(3, 128).T.copy(),
        b_g1=_np(b_g1).reshape(1, F), b_g2=_np(b_g2).reshape(1, F),
        b_o1=_np(b_o1).reshape(3, 128).T.copy(),
        b_o2=_np(b_o2).reshape(2, 1),
    )

    in_maps = []
    for c in range(NCORE):
        p = pad_pos[c * NPC:(c + 1) * NPC] - c * NPAD
        dT = np.zeros((DES, NPAD), mdt)
        dT[:, p] = des[c * NPC:(c + 1) * NPC].T
        nT = np.zeros((4, NPAD), mdt)
        nT[:, p] = num_prop[c * NPC:(c + 1) * NPC].T
        cT = np.zeros((3, NPAD), mdt)
        cT[:, p] = cat_prop[c * NPC:(c + 1) * NPC].T
        idx16, dstl, gcnt = per_core[c]
        in_maps.append(dict(
            desT=dT, numT=nT, catT=cT,
            deg1c=deg1_col[c],
            idx16=idx16, dstl=dstl, gcnt=gcnt, **shared))

    return nc, in_maps, pad_pos


def unshard(results, pad_pos):
    out = np.empty((N, 2), np.float32)
    for c in range(NCORE):
        o = results[c]["out2"]  # [2, NPAD]
        p = pad_pos[c * NPC:(c + 1) * NPC] - c * NPAD
        out[c * NPC:(c + 1) * NPC] = o[:, p].T
    return out


def kernel(**inputs):
    nc, in_maps, pad_pos = prepare(**inputs)
    res = run_bass_kernel_spmd(nc, in_maps, core_ids=list(range(NCORE)))
    return unshard(res.results, pad_pos)
